# revision 21
# baseline (speedup 1.0000x reference)
"""Multi-head attention (16 heads, L=2312, E=1024) on 8 trn2 NeuronCores.

Sharding: tensor-parallel over heads — each core computes 2 heads' full
attention (QKV proj + RoPE + softmax(QK^T)V), then an AllToAll re-shards
context from head-split to sequence-split so each core computes a disjoint
row-shard of the output projection. Host concatenates row shards.

Numerics: bf16 operands with fp32 PSUM accumulation + fp32 softmax
(exp / sum / reciprocal in fp32).

Self-contained: all shapes hardcoded; takes full unsharded inputs.
"""
import numpy as np
import ml_dtypes

import concourse.bacc as bacc
import concourse.tile as tile
from concourse import mybir
from concourse.bass_utils import run_bass_kernel_spmd
from concourse.masks import make_identity

N_CORES = 8
L = 2312           # valid sequence length
LP = 2432          # padded to 19*128
NK = LP // 128     # 19 key tiles
E = 1024
KE = E // 128      # 8 contraction tiles over embed dim
SHARD = LP // N_CORES  # 304 rows of output per core
F32 = mybir.dt.float32
BF16 = mybir.dt.bfloat16
SCALE = 0.125      # 1/sqrt(64)

# lq blocks: (start, width); widths multiples of 128 except last (2312-2048=264)
LQB = [(0, 512), (512, 512), (1024, 512), (1536, 512), (2048, 264)]
# qkv N blocks over padded seq
NBLK = [(0, 256), (256, 256), (512, 512), (1024, 512), (1536, 512), (2048, 384)]

_NC_CACHE = {}


def _build():
    if "nc" in _NC_CACHE:
        return _NC_CACHE["nc"]
    nc = bacc.Bacc(
        "TRN2",
        target_bir_lowering=False,
        debug=False,
        enable_asserts=False,
        num_devices=N_CORES,
    )
    xT = nc.dram_tensor("xT", [E, LP], BF16, kind="ExternalInput").ap()
    wT = nc.dram_tensor("wT", [E, 384], BF16, kind="ExternalInput").ap()
    bqkv = nc.dram_tensor("bqkv", [128, 3], F32, kind="ExternalInput").ap()
    cosT = nc.dram_tensor("cosT", [128, LP], BF16, kind="ExternalInput").ap()
    sinT = nc.dram_tensor("sinT", [128, LP], BF16, kind="ExternalInput").ap()
    mskT = nc.dram_tensor("mskT", [128, NK], F32, kind="ExternalInput").ap()
    pwT = nc.dram_tensor("pwT", [E, E], BF16, kind="ExternalInput").ap()
    pb = nc.dram_tensor("pb", [128, KE], F32, kind="ExternalInput").ap()
    perm = nc.dram_tensor("perm", [128, 128], BF16, kind="ExternalInput").ap()
    outT = nc.dram_tensor("outT", [E, SHARD], F32, kind="ExternalOutput").ap()

    with tile.TileContext(nc) as tc:
        with (
            tc.tile_pool(name="const", bufs=1) as cpool,
            tc.tile_pool(name="dram", bufs=1, space="DRAM") as dpool,
            tc.tile_pool(name="qkv", bufs=1) as qkvpool,
            tc.tile_pool(name="vaugp", bufs=1) as vaugpool,
            tc.tile_pool(name="ctxp", bufs=1) as ctxpool,
            tc.tile_pool(name="psb", bufs=6) as pspool,
            tc.tile_pool(name="ct", bufs=2) as ctpool,
            tc.tile_pool(name="cn", bufs=2) as cnpool,
            tc.tile_pool(name="rp", bufs=2) as rpool,
            tc.tile_pool(name="ps_c", bufs=1, space="PSUM") as psc,
        ):
            ident = cpool.tile([128, 128], F32)
            make_identity(nc, ident[:])
            identb = cpool.tile([128, 128], BF16)
            make_identity(nc, identb[:])
            pbias = cpool.tile([128, KE], F32)
            nc.sync.dma_start(pbias[:], pb)
            mask_sb = cpool.tile([128, NK], F32)
            nc.sync.dma_start(mask_sb[:], mskT)
            perm_sb = cpool.tile([128, 128], BF16)
            nc.sync.dma_start(perm_sb[:], perm)

            Q = qkvpool.tile([128, LP], BF16)
            Qz0 = qkvpool.tile([128, LP], BF16)  # [rope(q_h0) ; 0]
            Qz1 = qkvpool.tile([128, LP], BF16)  # [0 ; rope(q_h1)]
            K = qkvpool.tile([128, LP], BF16)
            V = qkvpool.tile([128, LP], BF16)
            vaug = vaugpool.tile([128, NK, 130], BF16)
            ctxTn = ctxpool.tile([128, LP], BF16)
            cc_in = dpool.tile([N_CORES, 128, SHARD], BF16)
            cc_out = dpool.tile([N_CORES, 128, SHARD], BF16)

            # ---------------- Phase A: QKV projection + RoPE + V transpose ----
            with (
                tc.tile_pool(name="xw", bufs=1) as xwpool,
                tc.tile_pool(name="ropet", bufs=3) as rtp,
                tc.tile_pool(name="ps_a", bufs=2, space="PSUM") as psa,
                tc.tile_pool(name="ps_sw", bufs=1, space="PSUM") as psw,
                tc.tile_pool(name="ps_vt", bufs=1, space="PSUM") as psvt,
                tc.tile_pool(name="ps_s0", bufs=1, space="PSUM") as spA,
            ):
                x_sb = xwpool.tile([128, KE, LP], BF16)
                w_sb = xwpool.tile([128, KE, 384], BF16)
                xTr = xT.rearrange("(k p) n -> p k n", p=128)
                wTr = wT.rearrange("(k p) m -> p k m", p=128)
                nc.sync.dma_start(w_sb[:, :, 0:128], wTr[:, :, 0:128])
                b_sb = xwpool.tile([128, 3], F32)
                nc.sync.dma_start(b_sb[:], bqkv)
                cos_sb = xwpool.tile([128, LP], BF16)
                sin_sb = xwpool.tile([128, LP], BF16)
                # x chunks n-major so the first matmuls start early; cos/sin
                # slot in after the first two x chunks
                for bi, (n0, nw) in enumerate(NBLK):
                    nc.sync.dma_start(x_sb[:, :, n0:n0 + nw], xTr[:, :, n0:n0 + nw])
                    if bi == 0:
                        nc.sync.dma_start(w_sb[:, :, 128:384], wTr[:, :, 128:384])
                    if bi == 1:
                        nc.sync.dma_start(cos_sb[:], cosT)
                        nc.sync.dma_start(sin_sb[:], sinT)

                # static zero halves of the zero-padded Q variants
                nc.any.memset(Qz0[64:128, :], 0.0)
                nc.any.memset(Qz1[0:64, :], 0.0)
                # mask columns of v_aug depend only on the mask DMA
                mview = mask_sb[:].rearrange("p (t o) -> p t o", o=1)
                nc.vector.tensor_copy(vaug[:, :, 64:65], mview)
                nc.vector.tensor_copy(vaug[:, :, 129:130], mview)

                def rope_chunk(T, n0, nw):
                    # rotate T[:, n0:n0+nw]; Q writes into the zero-padded
                    # per-head variants, K rotates in place. The 32-half swap
                    # within each head is a permutation matmul on PE.
                    swp = psw.tile([128, 512], F32, tag="swp", name=f"swp_{T.name}_{n0}")
                    nc.tensor.matmul(swp[:, :nw], perm_sb[:], T[:, n0:n0 + nw])
                    sw = rtp.tile([128, 512], BF16, tag="swap", name=f"sw_{T.name}_{n0}")
                    tmp = rtp.tile([128, 512], BF16, tag="tmp", name=f"tmp_{T.name}_{n0}")
                    nc.vector.tensor_mul(tmp[:, :nw], T[:, n0:n0 + nw], cos_sb[:, n0:n0 + nw])
                    nc.vector.tensor_mul(sw[:, :nw], swp[:, :nw], sin_sb[:, n0:n0 + nw])
                    if T is Q:
                        nc.vector.tensor_add(
                            Qz0[0:64, n0:n0 + nw], tmp[0:64, :nw], sw[0:64, :nw]
                        )
                        nc.vector.tensor_add(
                            Qz1[64:128, n0:n0 + nw], tmp[64:128, :nw], sw[64:128, :nw]
                        )
                    else:
                        nc.vector.tensor_add(T[:, n0:n0 + nw], tmp[:, :nw], sw[:, :nw])

                def vaug_chunk(n0, nw):
                    for t in range(n0 // 128, (n0 + nw) // 128):
                        tp = psvt.tile([128, 128], BF16, tag="vtp")
                        nc.tensor.transpose(tp[:], V[:, 128 * t:128 * (t + 1)], identb[:])
                        nc.vector.tensor_scalar_mul(
                            vaug[:, t, 0:64], tp[:, 0:64], mask_sb[:, t:t + 1]
                        )
                        nc.vector.tensor_scalar_mul(
                            vaug[:, t, 65:129], tp[:, 64:128], mask_sb[:, t:t + 1]
                        )

                # lq block 0's attention is interleaved into phase A so the
                # ScalarE exp stream starts as soon as inputs are ready
                QZ = [Qz0, Qz1]
                PC0 = [
                    psc.tile([65, 512], F32, tag=f"pc{h}", name=f"pc0{h}")
                    for h in range(2)
                ]
                att0_pending = []

                def att0_ctx(tp_, pb_):
                    for h in range(2):
                        nc.tensor.matmul(
                            PC0[h][:],
                            vaug[:, tp_, 65 * h:65 * h + 65],
                            pb_[:, 512 * h:512 * h + 512],
                            start=(tp_ == 0),
                            stop=(tp_ == NK - 1),
                        )

                def emit_att0(trange):
                    for t in trange:
                        SP = spA.tile([128, 1024], F32, tag="sp0", name="sp0")
                        PSb = pspool.tile([128, 1024], BF16, tag="psb", name="psb")
                        for h in range(2):
                            nc.tensor.matmul(
                                SP[:, 512 * h:512 * h + 512],
                                K[:, 128 * t:128 * (t + 1)],
                                QZ[h][:, 0:512],
                            )
                        nc.scalar.activation(
                            PSb[:], SP[:], mybir.ActivationFunctionType.Exp,
                            scale=SCALE,
                        )
                        att0_pending.append((t, PSb))
                        if len(att0_pending) >= 2:
                            att0_ctx(*att0_pending.pop(0))

                outs = [Q, K, V]
                _att0_hi = [0]
                for (n0, nw) in NBLK:
                    for m in range(3):
                        ps = psa.tile([128, 512], F32, tag="qkvps")
                        for k in range(KE):
                            nc.tensor.matmul(
                                ps[:, :nw],
                                w_sb[:, k, 128 * m:128 * (m + 1)],
                                x_sb[:, k, n0:n0 + nw],
                                start=(k == 0),
                                stop=(k == KE - 1),
                            )
                        nc.vector.tensor_scalar_add(
                            outs[m][:, n0:n0 + nw], ps[:, :nw], b_sb[:, m:m + 1]
                        )
                        if m < 2:
                            rope_chunk(outs[m], n0, nw)
                        else:
                            vaug_chunk(n0, nw)
                    # att0 reads Qz[:, 0:512]; only start once that is roped
                    if n0 + nw >= 512:
                        emit_att0(range(_att0_hi[0], (n0 + nw) // 128))
                        _att0_hi[0] = (n0 + nw) // 128
                for _pend in att0_pending:
                    att0_ctx(*_pend)
                att0_pending.clear()

            # ---------------- Phase B: attention per head ---------------------
            with tc.tile_pool(name="pw_ag", bufs=1) as pwpool:
                # proj weights load during phase B so the DMA engines are idle
                # when the AllToAll runs
                pw_sb = pwpool.tile([128, KE, E], BF16)
                pwr = pwT.rearrange("(k p) e -> p k e", p=128)
                nc.sync.dma_start(pw_sb[:, 0:4, :], pwr[:, 0:4, :])
                nc.sync.dma_start(pw_sb[:, 4:8, :], pwr[:, 4:8, :])
                with (
                    tc.tile_pool(name="ps_s", bufs=2, space="PSUM") as pss,
                    tc.tile_pool(name="ps_t", bufs=1, space="PSUM") as pst,
                ):
                    _cc_next = 0
                    first_block = True
                    for (lq0, lqw) in LQB:
                        if first_block:
                            # computed during phase A; just evict for the
                            # normalization path below
                            first_block = False
                            PCs = PC0
                        else:
                            PCs = [
                                psc.tile([65, 512], F32, tag=f"pc{h}", name=f"pc{h}")
                                for h in range(2)
                            ]
                        # software pipeline: ctx(t) is emitted after scores(t+1)
                        pending = []
                        for t in (range(0) if PCs is PC0 else range(NK)):
                            SP = pss.tile([128, 1024], F32, tag="sp", name="sp")
                            PSb = pspool.tile([128, 1024], BF16, tag="psb", name="psb")
                            for h in range(2):
                                nc.tensor.matmul(
                                    SP[:, 512 * h:512 * h + lqw],
                                    K[:, 128 * t:128 * (t + 1)],
                                    QZ[h][:, lq0:lq0 + lqw],
                                )
                            if lqw == 512:
                                nc.scalar.activation(
                                    PSb[:],
                                    SP[:],
                                    mybir.ActivationFunctionType.Exp,
                                    scale=SCALE,
                                )
                            else:
                                for h in range(2):
                                    nc.scalar.activation(
                                        PSb[:, 512 * h:512 * h + lqw],
                                        SP[:, 512 * h:512 * h + lqw],
                                        mybir.ActivationFunctionType.Exp,
                                        scale=SCALE,
                                    )
                            pending.append((t, PSb))
                            if len(pending) >= 2:
                                tp_, pb_ = pending.pop(0)
                                for h in range(2):
                                    nc.tensor.matmul(
                                        PCs[h][:, :lqw],
                                        vaug[:, tp_, 65 * h:65 * h + 65],
                                        pb_[:, 512 * h:512 * h + lqw],
                                        start=(tp_ == 0),
                                        stop=(tp_ == NK - 1),
                                    )
                        for tp_, pb_ in pending:
                            for h in range(2):
                                nc.tensor.matmul(
                                    PCs[h][:, :lqw],
                                    vaug[:, tp_, 65 * h:65 * h + 65],
                                    pb_[:, 512 * h:512 * h + lqw],
                                    start=(tp_ == 0),
                                    stop=(tp_ == NK - 1),
                                )
                        CTs = []
                        for h in range(2):
                            CT = ctpool.tile([65, 512], F32, tag=f"ct{h}", name=f"ct{h}")
                            nc.vector.tensor_copy(CT[:, :lqw], PCs[h][:, :lqw])
                            CTs.append(CT)
                        nsub = (lqw + 127) // 128
                        for s in range(nsub):
                            sw_ = min(128, lqw - 128 * s)
                            CN = cnpool.tile([128, 128], F32, tag="cn", name="cn")
                            for h in range(2):
                                TP1 = pst.tile([128, 65], F32, tag="tp1", name="tp1")
                                nc.tensor.transpose(
                                    TP1[:sw_, :],
                                    CTs[h][:, 128 * s:128 * s + sw_],
                                    ident[:65, :65],
                                )
                                Rc = rpool.tile([128, 1], F32, tag="rc", name="rc")
                                nc.vector.reciprocal(Rc[:sw_, :], TP1[:sw_, 64:65])
                                nc.vector.tensor_scalar_mul(
                                    CN[:sw_, 64 * h:64 * h + 64],
                                    TP1[:sw_, 0:64],
                                    Rc[:sw_, :],
                                )
                            TP2 = pst.tile([128, 128], F32, tag="tp2", name="tp2")
                            nc.tensor.transpose(
                                TP2[:, :sw_], CN[:sw_, :], ident[:sw_, :sw_]
                            )
                            nc.vector.tensor_copy(
                                ctxTn[:, lq0 + 128 * s:lq0 + 128 * s + sw_],
                                TP2[:, :sw_],
                            )
                        done = lq0 + lqw if lqw == 512 else LP
                        while _cc_next * SHARD + SHARD <= done:
                            j = _cc_next
                            nc.sync.dma_start(
                                cc_in[j], ctxTn[:, SHARD * j:SHARD * (j + 1)]
                            )
                            _cc_next += 1

                # ------------ Phase C: AllToAll re-shard + output projection --
                nc.gpsimd.collective_compute(
                    "AllToAll",
                    mybir.AluOpType.bypass,
                    replica_groups=[list(range(N_CORES))],
                    ins=[cc_in.opt()],
                    outs=[cc_out.opt()],
                )
                ag = pwpool.tile([128, KE, SHARD], BF16)
                ccr = cc_out[:].rearrange("j p n -> p j n")
                nc.sync.dma_start(ag[:, 0:2, :], ccr[:, 0:2, :])
                nc.sync.dma_start(ag[:, 2:8, :], ccr[:, 2:8, :])
                osb = pwpool.tile([128, KE, SHARD], F32)
                outTr = outT.rearrange("(k p) n -> p k n", p=128)
                with tc.tile_pool(name="ps_o", bufs=1, space="PSUM") as pso:
                    for half in (0, 1):
                        mEs = range(4 * half, 4 * half + 4)
                        pos = {
                            mE: pso.tile([128, 512], F32, tag=f"po{mE % 4}", name=f"po{mE}")
                            for mE in mEs
                        }
                        for k in range(KE):
                            for mE in mEs:
                                nc.tensor.matmul(
                                    pos[mE][:, :SHARD],
                                    pw_sb[:, k, 128 * mE:128 * (mE + 1)],
                                    ag[:, k, :],
                                    start=(k == 0),
                                    stop=(k == KE - 1),
                                )
                        for mE in mEs:
                            nc.vector.tensor_scalar_add(
                                osb[:, mE, :], pos[mE][:, :SHARD], pbias[:, mE:mE + 1]
                            )
                            nc.sync.dma_start(outTr[:, mE, :], osb[:, mE, :])

    nc.compile()
    _NC_CACHE["nc"] = nc
    return nc


def _prep_inputs(x, key_padding_mask, qkv_w, qkv_b, proj_w, proj_b, freqs_cos, freqs_sin):
    bf = ml_dtypes.bfloat16
    x = np.ascontiguousarray(np.asarray(x, np.float32))
    qkv_w = np.asarray(qkv_w, np.float32)
    qkv_b = np.asarray(qkv_b, np.float32)
    proj_w = np.asarray(proj_w, np.float32)
    proj_b = np.asarray(proj_b, np.float32)
    fc = np.asarray(freqs_cos, np.float32)  # [2304, 64]
    fs = np.asarray(freqs_sin, np.float32)
    mask = np.asarray(key_padding_mask)

    xT = np.zeros((E, LP), np.float32)
    xT[:, :L] = x.T
    xT = xT.astype(bf)

    cosT = np.ones((64, LP), np.float32)
    cosT[:, 8:L] = fc.T
    cos2 = np.concatenate([cosT, cosT], axis=0).astype(bf)  # [128, LP]

    sinT = np.zeros((64, LP), np.float32)
    sinT[:, 8:L] = fs.T
    sinT[:32, :] *= -1.0  # sign of -x2 half folded into sin table
    sin2 = np.concatenate([sinT, sinT], axis=0).astype(bf)

    maskf = np.zeros((LP,), np.float32)
    maskf[:L] = mask.astype(np.float32)
    mskT = np.ascontiguousarray(maskf.reshape(NK, 128).T)  # [128, NK]

    pwT = np.ascontiguousarray(proj_w.T).astype(bf)  # [d, e]
    permM = np.zeros((128, 128), np.float32)  # lhsT: permM[k, m]=1 iff k==swap(m)
    for m128 in range(128):
        swp = m128 + 32 if (m128 % 64) < 32 else m128 - 32
        permM[swp, m128] = 1.0
    permM = permM.astype(bf)
    pb2 = np.ascontiguousarray(proj_b.reshape(KE, 128).T)  # [128, KE]

    in_maps = []
    for c in range(N_CORES):
        h0, h1 = 2 * c, 2 * c + 1
        rows = []
        bias_rows = []
        for sec in range(3):  # q, k, v sections of qkv_w
            for h in (h0, h1):
                sl = slice(1024 * sec + 64 * h, 1024 * sec + 64 * h + 64)
                rows.append(qkv_w[sl])
                bias_rows.append(qkv_b[sl])
        Wc = np.concatenate(rows, axis=0)           # [384, 1024]
        bc = np.concatenate(bias_rows, axis=0)      # [384]
        in_maps.append({
            "xT": xT,
            "wT": np.ascontiguousarray(Wc.T).astype(bf),
            "bqkv": np.ascontiguousarray(bc.reshape(3, 128).T),
            "cosT": cos2,
            "sinT": sin2,
            "mskT": mskT,
            "pwT": pwT,
            "pb": pb2,
            "perm": permM,
        })
    return in_maps


def _run(in_maps, trace=False):
    nc = _build()
    return run_bass_kernel_spmd(
        nc, in_maps, core_ids=list(range(N_CORES)), trace=trace
    )


def kernel(x, key_padding_mask, qkv_w, qkv_b, proj_w, proj_b, freqs_cos, freqs_sin):
    in_maps = _prep_inputs(
        x, key_padding_mask, qkv_w, qkv_b, proj_w, proj_b, freqs_cos, freqs_sin
    )
    res = _run(in_maps, trace=False)
    outT_full = np.concatenate(
        [res.results[c]["outT"] for c in range(N_CORES)], axis=1
    )  # [E, LP]
    return np.ascontiguousarray(outT_full[:, :L].T).astype(np.float32)
